# revision 27
# baseline (speedup 1.0000x reference)
"""Trainium2 Bass kernel for BaseAttentionConvolution (7x7 neighborhood attention).

Computation (reference, fp32):
    q = Q @ Wq + bq                     # [B,H,W,64]
    k = K @ Wk + bk                     # [B,H,W,64]
    S[p, (dy,dx)] = q[p] . k[p+(dy,dx)]         (7x7 window, -inf outside image)
    P = softmax(S / 8)
    O[p] = sum_j P[p,j] * V[p+j]        # [B,H,W,128]
    out = relu(O @ Wv + bv)             # [B,H,W,128]

Sharding: B*H = 192 rows split into 8 bands of 24 rows (one per core).

Fast path (bq = bk = bv = 0, the shipped configuration), bf16 matmuls:
  - Host fuses G = Wq @ Wk^T so S = x_k^T G^T x_q needs no q/k projections:
    qG[128, 2304] = G^T-applied Q slab (on PE); scores contract raw kt
    against qG directly (kt is the stationary operand).
  - Bands of 4 query rows; for each of the 10 k-rows of a band only the
    valid query-row range is computed (widths 1,2,3,4,4,4,4,3,2,1 x 96),
    eliminating all redundant (q-row, k-row) pairs.
  - Per band the 10 score blocks are packed into two 3-PSUM-bank tiles
    of 1408 fp32 cols (no matmul crosses a bank) so exp and band-masking
    run as one big ACT/DVE op per phase instead of ten small ones.
  - HAM discipline: the PE clock gate (K/N throttle) demotes to 1.2 GHz
    after any ~idle window and re-promotes only under sustained busy.
    The kernel keeps the PE stall-free: early warmup matmuls on a
    memset-by-GPSIMD tile, qG chunks and PSUM-pad filler matmuls placed
    at the band-0 pipeline-priming points. The pad fillers double as the
    S-tile pad initializers (any finite value works: exp(pad) is killed
    by the bandp=0 mask), replacing DVE memsets.
  - Image-edge handling: K/V halo rows are zeros, so a halo row
    contributes exp(0)*band = band to the softmax denominator; a rank-1
    matmul subtracts the known count (-n_invalid(row) * bandwidth(x))
    from den. No kbias input, no per-row mask input.
  - Tail: den[1,384] is transposed in three 128-chunks (PE) so the
    reciprocal runs on 128 DVE lanes; oproj emits [qx,od] chunks and a
    single fused DVE tensor_scalar (mult by recip, max 0) does
    normalize+relu straight to bf16 for the output DMA.

Slow path (any nonzero bias): the original f32r kernel (v1) below.
"""

import numpy as np
from contextlib import ExitStack

import ml_dtypes

import concourse.bass as bass
import concourse.bacc as bacc
import concourse.tile as tile
from concourse import mybir
from concourse.bass_utils import run_bass_kernel_spmd

DT = mybir.dt.float32
BF = mybir.dt.bfloat16
FR = mybir.dt.float32r
AF = mybir.ActivationFunctionType
ALU = mybir.AluOpType
BF_NP = ml_dtypes.bfloat16

# Problem constants (hardcoded per contract)
B, H, W, C, KD, OD = 2, 96, 96, 128, 64, 128
KS, PAD = 7, 3
NCORES = 8
ROWS = (B * H) // NCORES        # 24 query rows per core
KROWS = ROWS + 2 * PAD          # 30 k/v rows per core (with halo)
NQ = ROWS * W                   # 2304 query pixels per core
NK = KROWS * W                  # 2880 key pixels per core
BAND = 4                        # query rows per band
NBANDS = ROWS // BAND           # 6
BN = BAND * W                   # 384 band query columns
NKR = BAND + 2 * PAD            # 10 k-rows per band
SCALE = 1.0 / np.sqrt(KD)       # 1/8
NWU = 11                        # HAM warmup matmuls

# Per-band k-row geometry: k-row i serves query rows c in [C0[i], C0[i]+WID[i])
C0 = [max(0, i - 2 * PAD) for i in range(NKR)]
WID = [min(BAND - 1, i) - max(0, i - 2 * PAD) + 1 for i in range(NKR)]
# Packed score-tile layout: two phases of five k-rows each, 3 PSUM banks
# (1408 fp32 cols); offsets chosen so no block crosses a 512-col bank edge.
PH_I = [(0, 1, 2, 3, 4), (5, 6, 7, 8, 9)]
OFF = {0: 384, 1: 800, 2: 512, 3: 0, 4: 1024,
       5: 0, 6: 1024, 7: 512, 8: 800, 9: 384}
SPACK = 1408
PADS = ((480, 512), (992, 1024))    # unused cols inside the packed layout


def build_nc():
    nc = bacc.Bacc(None, target_bir_lowering=False)
    qt = nc.dram_tensor("qt", [C, NQ], BF, kind="ExternalInput")
    kt = nc.dram_tensor("kt", [C, NK], BF, kind="ExternalInput")
    v = nc.dram_tensor("v", [W, KROWS, OD], BF, kind="ExternalInput")
    gw = nc.dram_tensor("gw", [C, C + OD], BF, kind="ExternalInput")
    bandp = nc.dram_tensor("bandp", [W, SPACK], BF, kind="ExternalInput")
    wcorr = nc.dram_tensor("wcorr", [1, NBANDS * BN], BF, kind="ExternalInput")
    out = nc.dram_tensor("out", [ROWS, W, OD], BF, kind="ExternalOutput")

    with tile.TileContext(nc) as tc, ExitStack() as ctx:
        consts = ctx.enter_context(tc.tile_pool(name="consts", bufs=1))
        slabs = ctx.enter_context(tc.tile_pool(name="slabs", bufs=1))
        e_pool = ctx.enter_context(tc.tile_pool(name="e_pool", bufs=2))
        e2_pool = ctx.enter_context(tc.tile_pool(name="e2_pool", bufs=2))
        sm_pool = ctx.enter_context(tc.tile_pool(name="sm_pool", bufs=2))
        ot_pool = ctx.enter_context(tc.tile_pool(name="ot_pool", bufs=2))
        os_pool = ctx.enter_context(tc.tile_pool(name="os_pool", bufs=2))
        ps_sa = ctx.enter_context(tc.tile_pool(name="ps_sa", bufs=1, space="PSUM"))
        ps_sb = ctx.enter_context(tc.tile_pool(name="ps_sb", bufs=1, space="PSUM"))
        ps_o = ctx.enter_context(tc.tile_pool(name="ps_o", bufs=1, space="PSUM"))
        ps_d = ctx.enter_context(tc.tile_pool(name="ps_d", bufs=1, space="PSUM"))

        # ---- input DMAs, need-ordered across two DGE rings.
        # scalar ring: kt c0 gates band 0's scores, bandp gates its mask.
        # sync ring: qt chunks gate qG; v chunks gate each band's ov. ----
        qt_s = slabs.tile([C, NQ], BF, tag="sqt")
        v_s = slabs.tile([W, KROWS, OD], BF, tag="sv")
        gw_s = consts.tile([C, C + OD], BF, tag="cgw")
        g_s = gw_s[:, :C]
        wv_s = gw_s[:, C : C + OD]
        nc.sync.dma_start(out=gw_s[:], in_=gw[:])
        nc.sync.dma_start(out=qt_s[:, 0:512], in_=qt[:, 0:512])
        nc.sync.dma_start(out=qt_s[:, 512:1536], in_=qt[:, 512:1536])
        nc.sync.dma_start(out=v_s[:, 0:12], in_=v[:, 0:12])
        nc.sync.dma_start(out=qt_s[:, 1536:NQ], in_=qt[:, 1536:NQ])
        nc.sync.dma_start(out=v_s[:, 12:KROWS], in_=v[:, 12:KROWS])
        kt_s = slabs.tile([C, NK], BF, tag="skt")
        nc.scalar.dma_start(out=kt_s[:, 0:960], in_=kt[:, 0:960])
        bandp_s = consts.tile([W, SPACK], BF, tag="cbp")
        nc.scalar.dma_start(out=bandp_s[:], in_=bandp[:])
        nc.scalar.dma_start(out=kt_s[:, 960:NK], in_=kt[:, 960:NK])

        # ---- constants. wraw is memset as the DVE's very first op so the
        # PE warm-up (which reads it) can start as soon as possible ----
        wraw = consts.tile([C, 512], BF, tag="cwraw")
        nc.vector.memset(wraw[:], 0.001)
        ones96 = consts.tile([W, 1], BF, tag="cones96")
        nc.vector.memset(ones96[:], 1.0)
        ones97 = consts.tile([W + 1, 1], BF, tag="cones97")
        nc.vector.memset(ones97[:], 1.0)
        ones1 = consts.tile([1, 1], DT, tag="cone1")
        nc.vector.memset(ones1[:], 1.0)

        # ---- engine warm-ups during the DMA wait: ACT exp-table preload,
        # GPSIMD tensor_tensor dispatch-path priming ----
        dummy = consts.tile([1, 1], DT, tag="cdum")
        nc.scalar.activation(dummy[:], gw_s[0:1, 0:1], AF.Exp, bias=0.0, scale=1.0)
        dum2 = consts.tile([1, 32], BF, tag="cdum2")
        nc.gpsimd.tensor_add(dum2[:], gw_s[0:1, 0:32], gw_s[0:1, 32:64])

        # ---- PE warm-up: dependency-free matmuls keep the HAM clock
        # gate's activity window busy from the moment the PE goes live;
        # the gate opens to 2.4 GHz ~3.4us later ----
        wu_ps = ps_sa.tile([C, SPACK], DT, tag="S")

        def warm(n, ps=None):
            tgt = wu_ps if ps is None else ps
            for _ in range(n):
                nc.tensor.matmul(
                    out=tgt[:, :512], lhsT=wraw[:, :C], rhs=wraw[:],
                    start=True, stop=True,
                )

        warm(7)

        def pad_fill(S):
            # PE writes junk into the packed-layout pad cols, making
            # exp(pad) finite; bandp=0 kills it. Only needed for the first
            # band: later S generations inherit stale-but-finite pads.
            for p0, p1 in PADS:
                nc.tensor.matmul(
                    out=S[:, p0:p1], lhsT=gw_s[:, :W], rhs=gw_s[:, : p1 - p0],
                    start=True, stop=True,
                )

        # ---- qG = G^T-applied Q slab: qG[:, p] = G^T @ q_pixel(p) ----
        qG_s = slabs.tile([C, NQ], BF, tag="sqG")
        qg_pools = {
            0: (ps_sb, "S"), 512: (ps_o, "outT"), 1024: (ps_d, "den"),
            1536: (ps_o, "outT"), 2048: (ps_d, "den"),
        }

        def qg_chunk(j0, eng):
            pool, tag = qg_pools[j0]
            j1 = min(j0 + 512, NQ)
            ps = pool.tile([C, j1 - j0], DT, tag=tag)
            nc.tensor.matmul(
                out=ps[:], lhsT=g_s[:], rhs=qt_s[:, j0:j1],
                start=True, stop=True,
            )
            if eng == "act":
                nc.scalar.copy(qG_s[:, j0:j1], ps[:])
            else:
                nc.vector.tensor_copy(qG_s[:, j0:j1], ps[:])

        qg_chunk(0, "dve")

        # ---- band loop, software-pipelined half a band deep: band P's
        # ph1 accumulation and tail are spread through band P+1 ----
        st = [dict() for _ in range(NBANDS)]

        def scores(band, ph):
            h0 = band * BAND
            S = (ps_sa if ph == 0 else ps_sb).tile([W, SPACK], DT, tag="S")
            for i in PH_I[ph]:
                r, o, w = h0 + i, OFF[i], WID[i]
                jq = slice((h0 + C0[i]) * W, (h0 + C0[i] + w) * W)
                nc.tensor.matmul(
                    out=S[:, o : o + w * W],
                    lhsT=kt_s[:, r * W : (r + 1) * W],
                    rhs=qG_s[:, jq],
                    start=True, stop=True,
                )
            if band == 0:
                pad_fill(S)
            E = e_pool.tile([W, SPACK], BF, tag="E")
            nc.scalar.activation(E[:], S[:], AF.Exp, bias=0.0, scale=SCALE)
            nc.vector.tensor_mul(E[:], E[:], bandp_s[:])
            st[band]["E%d" % ph] = E
            # pre-sum the two full-width blocks on the (idle) Pool engine:
            # halves the den matmul's column count for those blocks
            if ph == 0:
                e2 = e2_pool.tile([W, BN], BF, tag="e34")
                nc.gpsimd.tensor_add(e2[:], E[:, 0:BN], E[:, 1024 : 1024 + BN])
            else:
                e2 = e2_pool.tile([W + 1, BN], BF, tag="e56")
                nc.gpsimd.tensor_add(
                    e2[0:W, :], E[:, 0:BN], E[:, 1024 : 1024 + BN]
                )
                # fold the image-edge den correction in as a 97th row,
                # landed by a tiny DMA (no engine time)
                nc.scalar.dma_start(
                    out=e2[W : W + 1, :],
                    in_=wcorr[:, band * BN : (band + 1) * BN],
                )
            st[band]["e2%d" % ph] = e2

        def accum(band, ph):
            # ov accumulation: the full-width i=3 block is issued first with
            # start=True so every later sub-range write is pure accumulation
            h0 = band * BAND
            E = st[band]["E%d" % ph]
            e2 = st[band]["e2%d" % ph]
            outT, den = st[band]["outT"], st[band]["den"]
            order = (3, 0, 1, 2, 4) if ph == 0 else PH_I[1]
            for i in order:
                r, o, w = h0 + i, OFF[i], WID[i]
                js = slice(C0[i] * W, (C0[i] + w) * W)
                nc.tensor.matmul(
                    out=outT[:, js],
                    lhsT=v_s[:, r, :],
                    rhs=E[:, o : o + w * W],
                    start=(ph == 0 and i == 3), stop=(i == NKR - 1),
                )
            # den: pre-summed full-width pair + the three narrow blocks
            if ph == 0:
                nc.tensor.matmul(
                    out=den[:], lhsT=ones96[:], rhs=e2[:],
                    start=True, stop=False,
                )
            else:
                nc.tensor.matmul(
                    out=den[:], lhsT=ones97[:], rhs=e2[:],
                    start=False, stop=False,
                )
            narrow = (0, 1, 2) if ph == 0 else (7, 8, 9)
            for i in narrow:
                o, w = OFF[i], WID[i]
                js = slice(C0[i] * W, (C0[i] + w) * W)
                nc.tensor.matmul(
                    out=den[:, js],
                    lhsT=ones96[:],
                    rhs=E[:, o : o + w * W],
                    start=False, stop=(ph == 1 and i == 9),
                )

        def tail_feed(P):
            den_sb = sm_pool.tile([1, BN], DT, tag="densb")
            nc.scalar.copy(den_sb[:], st[P]["den"][:])
            oT = ot_pool.tile([OD, BN], BF, tag="oT")
            nc.vector.tensor_copy(oT[:], st[P]["outT"][:])
            st[P]["den_sb"], st[P]["oT"] = den_sb, oT

        def tail_recip(P):
            # den chunks transposed on PE so the reciprocal runs on 128 lanes
            denQ = ps_o.tile([OD, 3], DT, tag="outT")
            den_sb = st[P]["den_sb"]
            for j in range(3):
                nc.tensor.transpose(
                    denQ[:, j : j + 1], den_sb[:, j * OD : (j + 1) * OD], ones1[:]
                )
            recipQ = sm_pool.tile([OD, 3], DT, tag="recipQ")
            nc.vector.reciprocal(recipQ[:], denQ[:])
            st[P]["recipQ"] = recipQ

        def tail_out(P):
            # out-proj matmuls + fused normalize/relu + store
            oT, recipQ = st[P]["oT"], st[P]["recipQ"]
            op3 = ps_d.tile([OD, 3 * OD], DT, tag="den")
            ost = os_pool.tile([OD, 3 * OD], BF, tag="ost")
            for j in range(3):
                nc.tensor.matmul(
                    out=op3[:, j * OD : (j + 1) * OD],
                    lhsT=oT[:, j * OD : (j + 1) * OD],
                    rhs=wv_s[:],
                    start=True, stop=True,
                )
            nc.vector.scalar_tensor_tensor(
                out=ost[:].rearrange("p (c e) -> p c e", c=3),
                in0=op3[:].rearrange("p (c e) -> p c e", c=3),
                scalar=0.0,
                in1=recipQ[:, :, None].broadcast_to([OD, 3, OD]),
                op0=ALU.max,
                op1=ALU.mult,
            )
            h0p = P * BAND
            nc.sync.dma_start(
                out=out[h0p : h0p + BAND]
                .rearrange("r x e -> (r x) e")
                .rearrange("(c p) e -> p c e", c=3),
                in_=ost[:].rearrange("p (c e) -> p c e", c=3),
            )

        for band in range(NBANDS):
            P = band - 1
            scores(band, 0)
            if P >= 0:
                accum(P, 1)
                tail_feed(P)
            scores(band, 1)
            if band == 0:
                qg_chunk(512, "dve")
                qg_chunk(1024, "dve")
                # cover the exp0/mask0 latency before band 0's accum: the
                # PE has no prior-band tail work yet, so burn filler here
                wu_ps2 = ps_o.tile([C, 512], DT, tag="outT")
                warm(10, ps=wu_ps2)
            if P >= 0:
                tail_recip(P)
                tail_out(P)
            if band == 1:
                qg_chunk(1536, "act")
                qg_chunk(2048, "dve")
            outT = ps_o.tile([OD, BN], DT, tag="outT")
            den = ps_d.tile([1, BN], DT, tag="den")
            st[band]["outT"], st[band]["den"] = outT, den
            accum(band, 0)

        # final band's flush; filler matmuls keep the HAM gate open while
        # the PE waits on the tail's ACT/DVE hops
        Pl = NBANDS - 1
        accum(Pl, 1)
        tail_feed(Pl)
        wu_ps3 = ps_sb.tile([C, SPACK], DT, tag="S")
        warm(3, ps=wu_ps3)
        tail_recip(Pl)
        warm(2, ps=wu_ps3)
        tail_out(Pl)

    nc.compile()
    return nc


def _bf(x):
    return np.ascontiguousarray(np.asarray(x, np.float32)).astype(BF_NP)


def make_in_maps(Q, K, V, Wq, bq, Wk, bk, Wv, bv):
    Q = np.asarray(Q, np.float32)
    K = np.asarray(K, np.float32)
    V = np.asarray(V, np.float32)
    G = np.asarray(Wq, np.float32) @ np.asarray(Wk, np.float32).T  # [C, C]
    gw = np.concatenate([G, np.asarray(Wv, np.float32)], axis=1)  # [C, C+OD]
    gwb = _bf(gw)

    # band mask constant, packed-layout [96, 1408]
    idx = np.arange(W)
    band96 = (np.abs(idx[:, None] - idx[None, :]) <= PAD).astype(np.float32)
    bandp = np.zeros((W, SPACK), np.float32)
    for i in PH_I[0]:
        o, w = OFF[i], WID[i]
        bandp[:, o : o + w * W] = np.tile(band96, (1, w))
    bandp = _bf(bandp)

    bw = (np.minimum(idx + PAD, W - 1) - np.maximum(idx - PAD, 0) + 1).astype(
        np.float32
    )  # valid kx count per x

    in_maps = []
    for core in range(NCORES):
        b = core // (H // ROWS)
        h_start = (core % (H // ROWS)) * ROWS

        qs = Q[b, h_start : h_start + ROWS]              # [24,96,128]
        qtc = _bf(qs.reshape(NQ, C).T)                   # [128,2304]

        kpad = np.zeros((KROWS, W, C), np.float32)
        vpad = np.zeros((KROWS, W, C), np.float32)
        for j in range(KROWS):
            gr = h_start - PAD + j
            if 0 <= gr < H:
                kpad[j] = K[b, gr]
                vpad[j] = V[b, gr]
        ktc = _bf(kpad.reshape(NK, C).T)                 # [128,2880]
        vtc = _bf(vpad.transpose(1, 0, 2))               # [96,30,128]

        wcorr = np.zeros((1, NBANDS * BN), np.float32)
        for band in range(NBANDS):
            for c in range(BAND):
                gr = h_start + band * BAND + c
                n_inv = sum(
                    1 for dy in range(-PAD, PAD + 1) if not (0 <= gr + dy < H)
                )
                if n_inv:
                    wcorr[0, band * BN + c * W : band * BN + (c + 1) * W] = -n_inv * bw
        in_maps.append(
            {
                "qt": qtc,
                "kt": ktc,
                "v": vtc,
                "gw": gwb,
                "bandp": bandp,
                "wcorr": _bf(wcorr),
            }
        )
    return in_maps


def gather(results):
    full = np.empty((B, H, W, OD), np.float32)
    for core in range(NCORES):
        b = core // (H // ROWS)
        h_start = (core % (H // ROWS)) * ROWS
        full[b, h_start : h_start + ROWS] = np.asarray(
            results[core]["out"], np.float32
        )
    return full


_NC_CACHE = {}


def get_nc(path="v2"):
    if path not in _NC_CACHE:
        _NC_CACHE[path] = build_nc() if path == "v2" else build_nc_v1(
            with_bv=(path == "v1bv")
        )
    return _NC_CACHE[path]


def kernel(Q, K, V, Wq, bq, Wk, bk, Wv, bv):
    if np.any(np.asarray(bq)) or np.any(np.asarray(bk)):
        nc = get_nc("v1bv" if np.any(np.asarray(bv)) else "v1")
        in_maps = make_in_maps_v1(Q, K, V, Wq, bq, Wk, bk, Wv, bv)
    elif np.any(np.asarray(bv)):
        nc = get_nc("v1bv")
        in_maps = make_in_maps_v1(Q, K, V, Wq, bq, Wk, bk, Wv, bv)
    else:
        nc = get_nc("v2")
        in_maps = make_in_maps(Q, K, V, Wq, bq, Wk, bk, Wv, bv)
    res = run_bass_kernel_spmd(nc, in_maps, list(range(NCORES)))
    return gather(res.results)


# ======================================================================
# v1 fallback (original f32r kernel) — used only when a bias is nonzero.
# ======================================================================

WVN = 2 * OD
NEG = -30000.0


def build_nc_v1(with_bv=False):
    MDT = FR
    nc = bacc.Bacc(None, target_bir_lowering=False)
    qt = nc.dram_tensor("qt", [C, NQ], MDT, kind="ExternalInput")
    kt = nc.dram_tensor("kt", [C, NK], MDT, kind="ExternalInput")
    v = nc.dram_tensor("v", [W, KROWS, C], MDT, kind="ExternalInput")
    wq = nc.dram_tensor("wq", [C, KD], MDT, kind="ExternalInput")
    wk = nc.dram_tensor("wk", [C, KD], MDT, kind="ExternalInput")
    wv = nc.dram_tensor("wv", [C, WVN], MDT, kind="ExternalInput")
    bq = nc.dram_tensor("bq", [KD, 1], DT, kind="ExternalInput")
    bk = nc.dram_tensor("bk", [KD, 1], DT, kind="ExternalInput")
    bv = nc.dram_tensor("bv", [1, WVN], MDT, kind="ExternalInput")
    kbias = nc.dram_tensor("kbias", [W, KROWS], DT, kind="ExternalInput")
    ones_in = nc.dram_tensor("ones", [W, 1], MDT, kind="ExternalInput")
    b4 = nc.dram_tensor("b4", [W, NKR * BN], DT, kind="ExternalInput")
    out = nc.dram_tensor("out", [ROWS, W, OD], DT, kind="ExternalOutput")

    with tile.TileContext(nc) as tc, ExitStack() as ctx:
        consts = ctx.enter_context(tc.tile_pool(name="consts", bufs=1))
        slabs = ctx.enter_context(tc.tile_pool(name="slabs", bufs=1))
        e_pool = ctx.enter_context(tc.tile_pool(name="e_pool", bufs=3))
        o_pool = ctx.enter_context(tc.tile_pool(name="o_pool", bufs=2))
        r_pool = ctx.enter_context(tc.tile_pool(name="r_pool", bufs=2))
        rs_pool = ctx.enter_context(tc.tile_pool(name="rs_pool", bufs=8))
        outs = ctx.enter_context(tc.tile_pool(name="outs", bufs=3))
        ps_a = ctx.enter_context(tc.tile_pool(name="ps_a", bufs=3, space="PSUM"))
        ps_b = ctx.enter_context(tc.tile_pool(name="ps_b", bufs=2, space="PSUM"))
        ps_c = ctx.enter_context(tc.tile_pool(name="ps_c", bufs=2, space="PSUM"))

        wq_s = consts.tile([C, KD], MDT, tag="cw")
        nc.sync.dma_start(out=wq_s[:], in_=wq[:])
        wk_s = consts.tile([C, KD], MDT, tag="cw2")
        nc.sync.dma_start(out=wk_s[:], in_=wk[:])
        wv_s = consts.tile([C, WVN], MDT, tag="cw3")
        nc.sync.dma_start(out=wv_s[:], in_=wv[:])
        bq_s = consts.tile([KD, 1], DT, tag="cb")
        nc.sync.dma_start(out=bq_s[:], in_=bq[:])
        bk_s = consts.tile([KD, 1], DT, tag="cb2")
        nc.sync.dma_start(out=bk_s[:], in_=bk[:])
        kbias_s = consts.tile([W, KROWS], DT, tag="ckb")
        nc.sync.dma_start(out=kbias_s[:], in_=kbias[:])
        b4_s = consts.tile([W, NKR * BN], DT, tag="cb4")
        nc.sync.dma_start(out=b4_s[:], in_=b4[:])
        ones96 = consts.tile([W, 1], MDT, tag="cones")
        nc.sync.dma_start(out=ones96[:], in_=ones_in[:])
        ones1 = consts.tile([1, 1], DT, tag="cone1")
        nc.vector.memset(ones1[:], 1.0)
        if with_bv:
            bv_s = consts.tile([1, WVN], MDT, tag="cbv")
            nc.sync.dma_start(out=bv_s[:], in_=bv[:])

        qt_s = slabs.tile([C, NQ], MDT, tag="sqt")
        nc.sync.dma_start(out=qt_s[:], in_=qt[:])
        kt_s = slabs.tile([C, NK], MDT, tag="skt")
        nc.sync.dma_start(out=kt_s[:], in_=kt[:])
        v_s = slabs.tile([W, KROWS, C], MDT, tag="sv")
        nc.scalar.dma_start(out=v_s[:], in_=v[:])

        qT_s = slabs.tile([KD, NQ], MDT, tag="sqT")
        kT_s = slabs.tile([KD, NK], MDT, tag="skT")
        for dst, src, wmat, bvec, n in (
            (qT_s, qt_s, wq_s, bq_s, NQ),
            (kT_s, kt_s, wk_s, bk_s, NK),
        ):
            for j0 in range(0, n, 512):
                j1 = min(j0 + 512, n)
                ps = ps_a.tile([KD, 512], DT, tag="w")
                nc.tensor.matmul(
                    out=ps[:, : j1 - j0], lhsT=wmat[:], rhs=src[:, j0:j1],
                    start=True, stop=True,
                )
                nc.scalar.activation(
                    dst[:, j0:j1], ps[:, : j1 - j0], AF.Identity,
                    bias=bvec[:], scale=1.0,
                )

        for band in range(NBANDS):
            h0 = band * BAND
            jq = slice(h0 * W, (h0 + BAND) * W)
            outT = ps_b.tile([OD, BN], DT, tag="outT")
            den = ps_c.tile([1, BN], DT, tag="den")
            for i in range(NKR):
                r = h0 + i
                S = ps_a.tile([W, BN], DT, tag="w")
                nc.tensor.matmul(
                    out=S[:], lhsT=kT_s[:, r * W : (r + 1) * W],
                    rhs=qT_s[:, jq], start=True, stop=True,
                )
                E = e_pool.tile([W, BN], MDT, tag="E")
                nc.scalar.activation(
                    E[:], S[:], AF.Exp, bias=kbias_s[:, r : r + 1], scale=SCALE
                )
                nc.vector.tensor_mul(E[:], E[:], b4_s[:, i * BN : (i + 1) * BN])
                nc.tensor.matmul(
                    out=outT[:], lhsT=v_s[:, r, :], rhs=E[:],
                    start=(i == 0), stop=(i == NKR - 1),
                )
                nc.tensor.matmul(
                    out=den[:], lhsT=ones96[:], rhs=E[:],
                    start=(i == 0), stop=(i == NKR - 1),
                )

            recip = r_pool.tile([1, BN], DT, tag="recip")
            nc.vector.reciprocal(recip[:], den[:])
            oT = o_pool.tile([OD, BN], MDT, tag="oT")
            nc.vector.tensor_copy(oT[:], outT[:])
            if with_bv:
                den_sb = r_pool.tile([1, BN], MDT, tag="densb")
                nc.vector.tensor_copy(den_sb[:], den[:])
            for c in range(BAND):
                cs = slice(c * W, (c + 1) * W)
                rT = ps_a.tile([W, 1], DT, tag="w")
                nc.tensor.transpose(rT[:], recip[:, cs], ones1[:])
                rS = rs_pool.tile([W, 1], DT, tag="rS")
                nc.vector.tensor_copy(rS[:], rT[:])
                op = ps_a.tile([W, WVN], DT, tag="w")
                nc.tensor.matmul(
                    out=op[:], lhsT=oT[:, cs], rhs=wv_s[:],
                    start=True, stop=not with_bv,
                )
                if with_bv:
                    nc.tensor.matmul(
                        out=op[:], lhsT=den_sb[:, cs], rhs=bv_s[:],
                        start=False, stop=True,
                    )
                ost = outs.tile([W, OD], DT, tag="ost")
                nc.scalar.activation(ost[:], op[:, :OD], AF.Relu, bias=0.0, scale=rS[:])
                nc.sync.dma_start(out=out[h0 + c], in_=ost[:])

    nc.compile()
    return nc


def round_f32r(x):
    b = np.ascontiguousarray(x, np.float32).view(np.uint32)
    tie = (b >> 12) & 1
    b = (b + 0x7FF + tie) & np.uint32(0xFFFFF000)
    return b.view(np.float32)


def make_in_maps_v1(Q, K, V, Wq, bq, Wk, bk, Wv, bv):
    rnd = round_f32r
    Q = np.asarray(Q, np.float32)
    K = np.asarray(K, np.float32)
    V = np.asarray(V, np.float32)
    Wqr = rnd(np.asarray(Wq, np.float32))
    Wkr = rnd(np.asarray(Wk, np.float32))
    wvp = np.zeros((C, WVN), np.float32)
    wvp[:, :OD] = np.asarray(Wv, np.float32)
    wvp = rnd(wvp)
    bqv = np.ascontiguousarray(np.asarray(bq, np.float32).reshape(KD, 1))
    bkv = np.ascontiguousarray(np.asarray(bk, np.float32).reshape(KD, 1))
    bvp = np.zeros((1, WVN), np.float32)
    bvp[0, :OD] = np.asarray(bv, np.float32)
    bvp = rnd(bvp)

    idx = np.arange(W)
    b4 = (np.abs(idx[:, None] - idx[None, :]) <= PAD).astype(np.float32)
    b4i = np.zeros((W, NKR, BAND, W), np.float32)
    for i in range(NKR):
        for c in range(BAND):
            if i - 2 * PAD <= c <= i:
                b4i[:, i, c, :] = b4
    b4rep = np.ascontiguousarray(b4i.reshape(W, NKR * BAND * W))

    in_maps = []
    for core in range(NCORES):
        b = core // (H // ROWS)
        h_start = (core % (H // ROWS)) * ROWS
        qs = Q[b, h_start : h_start + ROWS]
        qtc = rnd(np.ascontiguousarray(qs.reshape(NQ, C).T))
        kpad = np.zeros((KROWS, W, C), np.float32)
        vpad = np.zeros((KROWS, W, C), np.float32)
        kb = np.full((KROWS,), NEG, np.float32)
        for j in range(KROWS):
            gr = h_start - PAD + j
            if 0 <= gr < H:
                kpad[j] = K[b, gr]
                vpad[j] = V[b, gr]
                kb[j] = 0.0
        ktc = rnd(np.ascontiguousarray(kpad.reshape(NK, C).T))
        vtc = rnd(np.ascontiguousarray(vpad.transpose(1, 0, 2)))
        kbias = np.ascontiguousarray(np.broadcast_to(kb[None, :], (W, KROWS)))
        in_maps.append(
            {
                "qt": qtc, "kt": ktc, "v": vtc,
                "wq": Wqr, "wk": Wkr, "wv": wvp,
                "bq": bqv, "bk": bkv, "bv": bvp,
                "kbias": kbias,
                "ones": np.ones((W, 1), np.float32),
                "b4": b4rep,
            }
        )
    return in_maps
